# revision 2
# baseline (speedup 1.0000x reference)
"""GCN encoder v2: dma_gather-based message fetch, fp16 h/messages, 8 cores.

Structure (one NEFF runs SPMD on 8 cores, so all shapes/offsets below are
core-uniform; only tensor *contents* differ per core):
  - Nodes are LPT-assigned (by in-degree) to 800 groups of 128 (100/core).
    Padded node id pid = group*128 + slot; core c owns groups [c*100,(c+1)*100).
  - h (the per-layer [N,128] fp16 transform output) is stored as quad-rows
    [25600, 512] (4 node rows per DRAM row) so a gather index fits in int16:
    node pid -> (q=pid//4, residue r=pid%4), gathered with elem_step=512
    elems (1024B) from column offset r*128.
  - Edges are bucketed by (core, supergroup sg=4 groups, r=src residue),
    sorted by dst group within the bucket, packed into 128-edge tiles.
    Tiles per (sg,r) = max over cores (uniform program, per-core padding
    with weight-0 slots). One dma_gather per (sg, r) fetches all of the
    bucket's source rows into SBUF.
  - Aggregation: per tile, for each dst group the tile touches on ANY core
    (union -> uniform), one PE matmul psum_g += sel^T @ msg where sel is a
    [128 edge, 128 dst-slot] one-hot*weight matrix built on DVE from
    host-packed (dstslot, weight) columns; w=0 rows contribute nothing.
  - transform: h = x @ W on PE (lhsT = x^T built by PE-transpose; layer 0
    reads a host-pretransposed x^T directly). bias+relu on DVE.
  - AllGather (fp16, 6.6MB per core per layer) replicates h; 3 layers in one
    NEFF.
"""

import heapq
import sys

import numpy as np

sys.path.insert(0, "/opt/trn_rl_repo")

N_NODES = 100000
N_EDGES = 1600000
D = 128
L = 3
C = 8
P = 128
G_PC = 100          # groups per core
SG_PC = 25          # supergroups per core (4 groups each)
NGROUPS = C * G_PC  # 800
N_PC = G_PC * P     # padded nodes per core (12800)
NPAD = C * N_PC     # 102400
NQ = NPAD // 4      # quad rows (25600)


class Plan:
    """Core-uniform program structure + per-core packed arrays."""

    def __init__(self):
        self.T_sr = None      # [SG_PC][4] tiles per (sg, residue)
        self.tb = None        # [SG_PC][4] local tile col base within sg
        self.T_sg = None      # [SG_PC] total tiles in sg
        self.iofs = None      # [SG_PC][4] global tile offset (for idx cols)
        self.mm = None        # [SG_PC] list of (tile_col_local, gamma, start, stop)
        self.m_ofs = None     # [SG_PC] global matmul col offset
        self.NT = 0           # total tiles per core
        self.NM = 0           # total matmul cols per core
        self.T_sg_max = 0
        self.NM_sg_max = 0


def partition_graph(edge_src, edge_dst, edge_weight):
    n = N_NODES
    deg = np.bincount(edge_dst, minlength=n).astype(np.int64)
    order = np.argsort(-deg, kind="stable")

    group_of = np.empty(n, np.int64)
    slot_of = np.empty(n, np.int64)
    count = np.zeros(NGROUPS, np.int64)
    load = np.zeros(NGROUPS, np.int64)
    heap = [(0, g) for g in range(NGROUPS)]
    heapq.heapify(heap)
    for node in order:
        while True:
            ld, g = heapq.heappop(heap)
            if count[g] < P:
                break
        group_of[node] = g
        slot_of[node] = count[g]
        count[g] += 1
        load[g] = ld + deg[node]
        if count[g] < P:
            heapq.heappush(heap, (load[g], g))
    pid = group_of * P + slot_of

    ps = pid[edge_src]
    r = (ps % 4).astype(np.int64)
    q16 = (ps // 4).astype(np.int64)
    g = group_of[edge_dst]
    core = g // G_PC
    gl = g % G_PC
    sg = gl // 4
    gam = gl % 4
    dslot = slot_of[edge_dst]

    # bucket = (core, sg, r); sort edges by (bucket, gam) stable
    bucket = (core * SG_PC + sg) * 4 + r
    skey = bucket * 4 + gam
    eord = np.argsort(skey, kind="stable")
    bucket_s = bucket[eord]
    gam_s = gam[eord]

    nb = C * SG_PC * 4
    cnt = np.bincount(bucket_s, minlength=nb).reshape(C, SG_PC, 4)
    T_percore = -(-cnt // P)  # ceil
    T_sr = np.maximum(T_percore.max(axis=0), 1)  # [SG_PC, 4] uniform

    plan = Plan()
    plan.T_sr = T_sr
    plan.tb = np.zeros((SG_PC, 4), np.int64)
    plan.tb[:, 1:] = np.cumsum(T_sr[:, :3], axis=1)
    plan.T_sg = T_sr.sum(axis=1)
    tiles_flat = T_sr.reshape(-1)
    iofs_flat = np.concatenate([[0], np.cumsum(tiles_flat)[:-1]])
    plan.iofs = iofs_flat.reshape(SG_PC, 4)
    plan.NT = int(tiles_flat.sum())
    plan.T_sg_max = int(plan.T_sg.max())

    # gather calls chunked to <=MAXT tiles (1024 idxs) to fit the SWDGE
    # descriptor ring (carveout 1024-2048 descs; >=1536-idx calls fault)
    MAXT = 8
    plan.calls = []
    for s in range(SG_PC):
        cl = []
        for rr in range(4):
            T = int(T_sr[s, rr])
            base = int(plan.tb[s, rr])
            io = int(plan.iofs[s, rr])
            for o in range(0, T, MAXT):
                nt_ = min(MAXT, T - o)
                cl.append((base + o, io + o, nt_, rr))
        plan.calls.append(cl)

    # rank within bucket, per core-ordered edges
    starts = np.searchsorted(bucket_s, np.arange(nb))
    rank = np.arange(len(eord)) - starts[bucket_s]
    tloc = rank // P
    slot = rank % P
    assert (tloc < T_sr.reshape(-1)[bucket_s % (SG_PC * 4) + 0]).all() or True
    # (per-core T bound check below)
    core_s = bucket_s // (SG_PC * 4)
    sg_s = (bucket_s // 4) % SG_PC
    r_s = bucket_s % 4
    assert (tloc < T_sr[sg_s, r_s]).all()

    # gamma coverage per (sg, r, tile) across cores: sorted by gam within
    # bucket, so each (core, bucket, tile) spans a gam range [lo, hi].
    # Mark presence of each (sg, r, t, gam) with any edge on any core.
    T_max = int(T_sr.max())
    present = np.zeros((SG_PC, 4, T_max, 4), bool)
    present[sg_s, r_s, tloc, gam_s] = True

    plan.mm = []
    plan.m_ofs = []
    LK = np.full((SG_PC, 4, T_max, 4), -1, np.int64)
    m_total = 0
    NM_sg_max = 0
    for s in range(SG_PC):
        mlist = []
        covered = [False] * 4
        for rr in range(4):
            for t in range(int(T_sr[s, rr])):
                for gm in range(4):
                    if present[s, rr, t, gm]:
                        LK[s, rr, t, gm] = len(mlist)
                        mlist.append([int(plan.tb[s, rr] + t), gm, False, False])
                        covered[gm] = True
        for gm in range(4):
            if not covered[gm]:
                mlist.append([int(plan.tb[s, 0]), gm, False, False])
        # start/stop per gamma
        first = {}
        last = {}
        for j, (tc, gm, _, _) in enumerate(mlist):
            if gm not in first:
                first[gm] = j
            last[gm] = j
        for gm in range(4):
            mlist[first[gm]][2] = True
            mlist[last[gm]][3] = True
        plan.mm.append([tuple(x) for x in mlist])
        plan.m_ofs.append(m_total)
        m_total += len(mlist)
        NM_sg_max = max(NM_sg_max, len(mlist))
    plan.NM = m_total
    plan.NM_sg_max = NM_sg_max

    # per-edge global positions
    q16_s = q16[eord]
    dslot_s = dslot[eord]
    w_s = np.asarray(edge_weight, np.float32)[eord]
    I = (plan.iofs[sg_s, r_s] + tloc) * P + slot  # global idx position per core
    mcol = np.asarray(plan.m_ofs, np.int64)[sg_s] + LK[sg_s, r_s, tloc, gam_s]
    assert (LK[sg_s, r_s, tloc, gam_s] >= 0).all()

    idx_small = np.zeros((C, 16, plan.NT * 8), np.int16)
    idx_small[core_s, I % 16, I // 16] = q16_s.astype(np.int16)
    idx_arr = np.tile(idx_small, (1, 8, 1))
    dst_arr = np.zeros((C, P, plan.NM), np.float16)
    w_arr = np.zeros((C, P, plan.NM), np.float16)
    dst_arr[core_s, slot, mcol] = dslot_s.astype(np.float16)
    w_arr[core_s, slot, mcol] = w_s.astype(np.float16)

    return idx_arr, dst_arr, w_arr, pid, plan


def emulate(x, W, b, idx_arr, dst_arr, w_arr, pid, plan, fp16=True):
    """Numpy emulation of the planned device computation (for validation)."""
    cast = (lambda a: a.astype(np.float16).astype(np.float32)) if fp16 else (lambda a: a)
    x_sh = np.zeros((NPAD, D), np.float32)
    x_sh[pid] = x
    Wc = cast(np.asarray(W, np.float32))
    out = np.zeros((NPAD, D), np.float32)
    h = cast(x_sh) @ Wc[0]
    h = cast(h)  # stored fp16
    for l in range(L):
        hq = h.reshape(NQ, 4 * D)
        agg = np.zeros((NPAD, D), np.float32)
        for c in range(C):
            for s in range(SG_PC):
                # gather
                msg = np.zeros((plan.T_sg[s] * P, D), np.float32)
                for rr in range(4):
                    Tsr = int(plan.T_sr[s, rr])
                    io = plan.iofs[s, rr]
                    idxs = idx_arr[c, :16, io * 8:(io + Tsr) * 8]
                    flat = idxs.T.reshape(-1)  # (s p) order
                    rows = hq[flat.astype(np.int64), rr * D:(rr + 1) * D]
                    msg[plan.tb[s, rr] * P:(plan.tb[s, rr] + Tsr) * P] = rows
                psums = np.zeros((4, P, D), np.float32)
                mo = plan.m_ofs[s]
                for j, (tc, gm, st, sp) in enumerate(plan.mm[s]):
                    dstc = dst_arr[c, :, mo + j].astype(np.float32)
                    wc = w_arr[c, :, mo + j].astype(np.float32)
                    sel = (dstc[:, None] == np.arange(P)[None, :]) * wc[:, None]
                    psums[gm] += sel.T @ msg[tc * P:(tc + 1) * P]
                for gm in range(4):
                    gg = (c * G_PC) + s * 4 + gm
                    agg[gg * P:(gg + 1) * P] = psums[gm]
        xn = np.maximum(agg + b[l][None, :], 0.0)
        if l < L - 1:
            xn16 = cast(xn)
            h = cast(xn16 @ Wc[l + 1])
        else:
            out = xn
    return out[pid]


def build_nc(plan, queues=1, repeats=1, skip_gather=False, skip_ag=False, skip_sel=False):
    import concourse.bacc as bacc
    import concourse.mybir as mybir
    import concourse.tile as tile

    f32 = mybir.dt.float32
    f16 = mybir.dt.float16
    i16 = mybir.dt.int16
    NT, NM = plan.NT, plan.NM

    nc = bacc.Bacc(
        "TRN2",
        target_bir_lowering=False,
        debug=False,
        num_devices=C,
        num_swdge_queues=queues,
        dynamic_dma_scratch_size=32768,
    )

    xT_in = nc.dram_tensor("xT_in", [P, N_PC], f16, kind="ExternalInput")
    idx_in = nc.dram_tensor("idx_in", [P, NT * 8], i16, kind="ExternalInput")
    dst_in = nc.dram_tensor("dst_in", [P, NM], f16, kind="ExternalInput")
    w_in = nc.dram_tensor("w_in", [P, NM], f16, kind="ExternalInput")
    W_in = nc.dram_tensor("W_in", [L, D, D], f16, kind="ExternalInput")
    b_in = nc.dram_tensor("b_in", [L, P, D], f32, kind="ExternalInput")
    iota_in = nc.dram_tensor("iota_in", [P, P], f16, kind="ExternalInput")
    ident_in = nc.dram_tensor("ident_in", [P, P], f16, kind="ExternalInput")
    x_out = nc.dram_tensor("x_out", [N_PC, D], f32, kind="ExternalOutput")

    with tile.TileContext(nc) as tc:
        with (
            tc.tile_pool(name="const", bufs=1) as cpool,
            tc.tile_pool(name="work", bufs=3) as spool,
            tc.tile_pool(name="selp", bufs=2) as selpool,
            tc.tile_pool(name="msgp", bufs=2) as mpool,
            tc.tile_pool(name="psum", bufs=1, space="PSUM") as ppool,
            tc.tile_pool(name="psum2", bufs=2, space="PSUM") as ppool2,
            tc.tile_pool(name="dram", bufs=1, space="DRAM") as dpool,
        ):
            idx_sb = cpool.tile([P, NT * 8], i16, tag="idx")
            nc.sync.dma_start(out=idx_sb[:], in_=idx_in[:])
            dst_sb = cpool.tile([P, NM], f16, tag="dst")
            nc.sync.dma_start(out=dst_sb[:], in_=dst_in[:])
            w_sb = cpool.tile([P, NM], f16, tag="w")
            nc.sync.dma_start(out=w_sb[:], in_=w_in[:])
            W_sb = cpool.tile([P, L * D], f16, tag="W")
            for l in range(L):
                nc.sync.dma_start(out=W_sb[:, l * D:(l + 1) * D], in_=W_in[l])
            b_sb = cpool.tile([P, L * D], f32, tag="b")
            for l in range(L):
                nc.sync.dma_start(out=b_sb[:, l * D:(l + 1) * D], in_=b_in[l])
            iota_sb = cpool.tile([P, P], f16, tag="iota")
            nc.sync.dma_start(out=iota_sb[:], in_=iota_in[:])
            id_sb = cpool.tile([P, P], f16, tag="ident")
            nc.sync.dma_start(out=id_sb[:], in_=ident_in[:])

            h_loc = [
                dpool.tile([N_PC // 4, 4 * D], f16, tag=f"hloc{l}", name=f"hloc{l}")
                for l in range(L)
            ]
            h_full = [
                dpool.tile([NQ, 4 * D], f16, tag=f"hfull{l}", name=f"hfull{l}")
                for l in range(L)
            ]

            def transform(l, g, xT_t):
                """h_loc[l] rows for group g from xT_t = x^T [feat, node]."""
                psum_h = ppool2.tile([P, P], f32, tag="psum_h")
                nc.tensor.matmul(
                    out=psum_h[:],
                    lhsT=xT_t[:],
                    rhs=W_sb[:, l * D:(l + 1) * D],
                    start=True,
                    stop=True,
                )
                h_sb = spool.tile([P, P], f16, tag="h_sb")
                nc.vector.tensor_copy(out=h_sb[:], in_=psum_h[:])
                nc.sync.dma_start(
                    out=h_loc[l][g * 32:(g + 1) * 32, :], in_=h_sb[:]
                )

            def all_gather(l):
                if skip_ag:
                    nc.sync.dma_start(out=h_full[l][0:N_PC // 4, :],
                                      in_=h_loc[l][:, :])
                    return
                nc.gpsimd.collective_compute(
                    "AllGather",
                    mybir.AluOpType.bypass,
                    replica_groups=[list(range(C))],
                    ins=[h_loc[l][:, :]],
                    outs=[h_full[l][:, :]],
                )

            for _rep in range(repeats):
              for g in range(G_PC):
                xT_t = spool.tile([P, P], f16, tag="xT")
                nc.sync.dma_start(out=xT_t[:], in_=xT_in[:, g * P:(g + 1) * P])
                transform(0, g, xT_t)
              all_gather(0)

              for l in range(L):
                last = l == L - 1
                for s in range(SG_PC):
                    msg = mpool.tile([P, plan.T_sg_max * P], f16, tag="msg")
                    for ci, (lc, io, nt_, rr) in enumerate(plan.calls[s]):
                        out3 = msg[:, lc * P:(lc + nt_) * P].rearrange(
                            "p (c f) -> p c f", f=P
                        )
                        if skip_gather:
                            base = ((s * 31 + ci) * 977) % (NQ - nt_ * 32)
                            nc.sync.dma_start(
                                out=msg[:, lc * P:(lc + nt_) * P],
                                in_=h_full[l][base:base + nt_ * 32, :])
                            continue
                        nc.gpsimd.dma_gather(
                            out3,
                            h_full[l][:, rr * D:(rr + 1) * D],
                            idx_sb[:, io * 8:(io + nt_) * 8],
                            nt_ * P,
                            nt_ * P,
                            D,
                            elem_step=4 * D,
                            queue_num=ci % queues,
                        )
                    nm = len(plan.mm[s])
                    mo = plan.m_ofs[s]
                    sel = selpool.tile([P, plan.NM_sg_max * P], f16, tag="sel")
                    sel3 = sel[:, :nm * P].rearrange("p (m j) -> p m j", j=P)
                    dst3 = dst_sb[:, mo:mo + nm].to_broadcast([P, nm, P])
                    w3 = w_sb[:, mo:mo + nm].to_broadcast([P, nm, P])
                    iota3 = (
                        iota_sb[:]
                        .rearrange("p (o j) -> p o j", o=1)
                        .to_broadcast([P, nm, P])
                    )
                    nc.vector.tensor_tensor(
                        out=sel3, in0=dst3, in1=iota3, op=mybir.AluOpType.is_equal
                    )
                    if not skip_sel:
                        nc.vector.tensor_tensor(
                            out=sel3, in0=sel3, in1=w3, op=mybir.AluOpType.mult
                        )
                    paggs = [
                        ppool.tile([P, P], f32, tag=f"agg{gm}", name=f"agg{gm}")
                        for gm in range(4)
                    ]
                    for j, (tc_, gm, st, sp) in enumerate(plan.mm[s]):
                        nc.tensor.matmul(
                            out=paggs[gm][:],
                            lhsT=sel[:, j * P:(j + 1) * P],
                            rhs=msg[:, tc_ * P:(tc_ + 1) * P],
                            start=st,
                            stop=sp,
                        )
                    for gm in range(4):
                        g = s * 4 + gm
                        if last:
                            xo = spool.tile([P, P], f32, tag="xo")
                            nc.vector.tensor_tensor(
                                out=xo[:],
                                in0=paggs[gm][:],
                                in1=b_sb[:, l * D:(l + 1) * D],
                                op=mybir.AluOpType.add,
                            )
                            nc.vector.tensor_scalar_max(xo[:], xo[:], 0.0)
                            nc.sync.dma_start(
                                out=x_out[g * P:(g + 1) * P, :], in_=xo[:]
                            )
                        else:
                            xnew = spool.tile([P, P], f16, tag="xnew")
                            nc.vector.tensor_tensor(
                                out=xnew[:],
                                in0=paggs[gm][:],
                                in1=b_sb[:, l * D:(l + 1) * D],
                                op=mybir.AluOpType.add,
                            )
                            nc.vector.tensor_scalar_max(xnew[:], xnew[:], 0.0)
                            psum_t = ppool2.tile([P, P], f16, tag="psum_t")
                            nc.tensor.transpose(
                                out=psum_t[:], in_=xnew[:], identity=id_sb[:]
                            )
                            xT_t = spool.tile([P, P], f16, tag="xT")
                            nc.vector.tensor_copy(out=xT_t[:], in_=psum_t[:])
                            transform(l + 1, g, xT_t)
                if not last:
                    all_gather(l + 1)

    nc.compile()
    return nc


def make_host_inputs(x, edge_src, edge_dst, edge_weight, W, b):
    idx_arr, dst_arr, w_arr, pid, plan = partition_graph(
        edge_src, edge_dst, edge_weight
    )
    x_sh = np.zeros((NPAD, D), np.float32)
    x_sh[pid] = np.asarray(x, np.float32)
    xT = (
        x_sh.reshape(C, N_PC, D)
        .transpose(0, 2, 1)
        .astype(np.float16)
        .copy()
    )
    b_rep = np.broadcast_to(
        np.asarray(b, np.float32)[:, None, :], (L, P, D)
    ).copy()
    iota = np.broadcast_to(
        np.arange(P, dtype=np.float16), (P, P)
    ).copy()
    ident = np.eye(P, dtype=np.float16)
    W16 = np.asarray(W, np.float32).astype(np.float16)
    in_maps = [
        {
            "xT_in": xT[c],
            "idx_in": idx_arr[c],
            "dst_in": dst_arr[c],
            "w_in": w_arr[c],
            "W_in": W16,
            "b_in": b_rep,
            "iota_in": iota,
            "ident_in": ident,
        }
        for c in range(C)
    ]
    return in_maps, pid, plan


def unshard_output(results, pid):
    full = np.concatenate([results[c]["x_out"] for c in range(C)], axis=0)
    return full[pid]


QUEUES = 4


def run(x, edge_src, edge_dst, edge_weight, W, b, queues=None):
    if queues is None:
        queues = QUEUES
    from concourse.bass_utils import run_bass_kernel_spmd

    x = np.asarray(x, np.float32)
    edge_src = np.asarray(edge_src, np.int32)
    edge_dst = np.asarray(edge_dst, np.int32)
    edge_weight = np.asarray(edge_weight, np.float32)
    in_maps, pid, plan = make_host_inputs(x, edge_src, edge_dst, edge_weight, W, b)
    nc = build_nc(plan, queues=queues)
    res = run_bass_kernel_spmd(nc, in_maps, core_ids=list(range(C)))
    return unshard_output(res.results, pid), res


def kernel(x, edge_src, edge_dst, edge_weight, W, b):
    out, _ = run(x, edge_src, edge_dst, edge_weight, W, b)
    return out


# revision 3
# speedup vs baseline: 1.3807x; 1.3807x over previous
"""GCN encoder v2: dma_gather-based message fetch, fp16 h/messages, 8 cores.

Structure (one NEFF runs SPMD on 8 cores, so all shapes/offsets below are
core-uniform; only tensor *contents* differ per core):
  - Nodes are LPT-assigned (by in-degree) to 800 groups of 128 (100/core).
    Padded node id pid = group*128 + slot; core c owns groups [c*100,(c+1)*100).
  - h (the per-layer [N,128] fp16 transform output) is stored as quad-rows
    [25600, 512] (4 node rows per DRAM row) so a gather index fits in int16:
    node pid -> (q=pid//4, residue r=pid%4), gathered with elem_step=512
    elems (1024B) from column offset r*128.
  - Edges are bucketed by (core, supergroup sg=4 groups, r=src residue),
    sorted by dst group within the bucket, packed into 128-edge tiles.
    Tiles per (sg,r) = max over cores (uniform program, per-core padding
    with weight-0 slots). One dma_gather per (sg, r) fetches all of the
    bucket's source rows into SBUF.
  - Aggregation: per tile, for each dst group the tile touches on ANY core
    (union -> uniform), one PE matmul psum_g += sel^T @ msg where sel is a
    [128 edge, 128 dst-slot] one-hot*weight matrix built on DVE from
    host-packed (dstslot, weight) columns; w=0 rows contribute nothing.
  - transform: h = x @ W on PE (lhsT = x^T built by PE-transpose; layer 0
    reads a host-pretransposed x^T directly). bias+relu on DVE.
  - AllGather (fp16, 6.6MB per core per layer) replicates h; 3 layers in one
    NEFF.
"""

import heapq
import sys

import numpy as np

sys.path.insert(0, "/opt/trn_rl_repo")

N_NODES = 100000
N_EDGES = 1600000
D = 128
L = 3
C = 8
P = 128
G_PC = 100          # groups per core
SG_PC = 25          # supergroups per core (4 groups each)
NGROUPS = C * G_PC  # 800
N_PC = G_PC * P     # padded nodes per core (12800)
NPAD = C * N_PC     # 102400
NQ = NPAD // 4      # quad rows (25600)
CH0_G = 48          # groups per core in AllGather chunk 0 (SGs 0-11)
CH0Q = CH0_G * 32   # 1536 q-rows per core in chunk 0
CH1Q = N_PC // 4 - CH0Q


class Plan:
    """Core-uniform program structure + per-core packed arrays."""

    def __init__(self):
        self.T_sr = None      # [SG_PC][4] tiles per (sg, residue)
        self.tb = None        # [SG_PC][4] local tile col base within sg
        self.T_sg = None      # [SG_PC] total tiles in sg
        self.iofs = None      # [SG_PC][4] global tile offset (for idx cols)
        self.mm = None        # [SG_PC] list of (tile_col_local, gamma, start, stop)
        self.m_ofs = None     # [SG_PC] global matmul col offset
        self.NT = 0           # total tiles per core
        self.NM = 0           # total matmul cols per core
        self.T_sg_max = 0
        self.NM_sg_max = 0


def partition_graph(edge_src, edge_dst, edge_weight):
    n = N_NODES
    deg = np.bincount(edge_dst, minlength=n).astype(np.int64)
    order = np.argsort(-deg, kind="stable")

    group_of = np.empty(n, np.int64)
    slot_of = np.empty(n, np.int64)
    count = np.zeros(NGROUPS, np.int64)
    load = np.zeros(NGROUPS, np.int64)
    heap = [(0, g) for g in range(NGROUPS)]
    heapq.heapify(heap)
    for node in order:
        while True:
            ld, g = heapq.heappop(heap)
            if count[g] < P:
                break
        group_of[node] = g
        slot_of[node] = count[g]
        count[g] += 1
        load[g] = ld + deg[node]
        if count[g] < P:
            heapq.heappush(heap, (load[g], g))
    pid = group_of * P + slot_of

    ps = pid[edge_src]
    r = (ps % 4).astype(np.int64)
    q16 = (ps // 4).astype(np.int64)
    # chunk-major h_full layout: chunk 0 = [8 cores x 1536 q-rows] then
    # chunk 1 = [8 cores x 1664], so each split-AllGather output is contiguous
    c_src = q16 // (N_PC // 4)
    lq = q16 % (N_PC // 4)
    q16 = np.where(lq < CH0Q, c_src * CH0Q + lq,
                   C * CH0Q + c_src * CH1Q + (lq - CH0Q))
    g = group_of[edge_dst]
    core = g // G_PC
    gl = g % G_PC
    sg = gl // 4
    gam = gl % 4
    dslot = slot_of[edge_dst]

    # bucket = (core, sg, r); sort edges by (bucket, gam) stable
    bucket = (core * SG_PC + sg) * 4 + r
    skey = bucket * 4 + gam
    eord = np.argsort(skey, kind="stable")
    bucket_s = bucket[eord]
    gam_s = gam[eord]

    nb = C * SG_PC * 4
    cnt = np.bincount(bucket_s, minlength=nb).reshape(C, SG_PC, 4)
    T_percore = -(-cnt // P)  # ceil
    T_sr = np.maximum(T_percore.max(axis=0), 1)  # [SG_PC, 4] uniform

    plan = Plan()
    plan.T_sr = T_sr
    plan.tb = np.zeros((SG_PC, 4), np.int64)
    plan.tb[:, 1:] = np.cumsum(T_sr[:, :3], axis=1)
    plan.T_sg = T_sr.sum(axis=1)
    tiles_flat = T_sr.reshape(-1)
    iofs_flat = np.concatenate([[0], np.cumsum(tiles_flat)[:-1]])
    plan.iofs = iofs_flat.reshape(SG_PC, 4)
    plan.NT = int(tiles_flat.sum())
    plan.T_sg_max = int(plan.T_sg.max())

    # gather calls chunked to <=MAXT tiles (1024 idxs) to fit the SWDGE
    # descriptor ring (carveout 1024-2048 descs; >=1536-idx calls fault)
    MAXT = 8
    plan.calls = []
    for s in range(SG_PC):
        cl = []
        for rr in range(4):
            T = int(T_sr[s, rr])
            base = int(plan.tb[s, rr])
            io = int(plan.iofs[s, rr])
            for o in range(0, T, MAXT):
                nt_ = min(MAXT, T - o)
                cl.append((base + o, io + o, nt_, rr))
        plan.calls.append(cl)

    # rank within bucket, per core-ordered edges
    starts = np.searchsorted(bucket_s, np.arange(nb))
    rank = np.arange(len(eord)) - starts[bucket_s]
    tloc = rank // P
    slot = rank % P
    assert (tloc < T_sr.reshape(-1)[bucket_s % (SG_PC * 4) + 0]).all() or True
    # (per-core T bound check below)
    core_s = bucket_s // (SG_PC * 4)
    sg_s = (bucket_s // 4) % SG_PC
    r_s = bucket_s % 4
    assert (tloc < T_sr[sg_s, r_s]).all()

    # gamma coverage per (sg, r, tile) across cores: sorted by gam within
    # bucket, so each (core, bucket, tile) spans a gam range [lo, hi].
    # Mark presence of each (sg, r, t, gam) with any edge on any core.
    T_max = int(T_sr.max())
    present = np.zeros((SG_PC, 4, T_max, 4), bool)
    present[sg_s, r_s, tloc, gam_s] = True

    plan.mm = []
    plan.m_ofs = []
    LK = np.full((SG_PC, 4, T_max, 4), -1, np.int64)
    m_total = 0
    NM_sg_max = 0
    for s in range(SG_PC):
        mlist = []
        covered = [False] * 4
        for rr in range(4):
            for t in range(int(T_sr[s, rr])):
                for gm in range(4):
                    if present[s, rr, t, gm]:
                        LK[s, rr, t, gm] = len(mlist)
                        mlist.append([int(plan.tb[s, rr] + t), gm, False, False])
                        covered[gm] = True
        for gm in range(4):
            if not covered[gm]:
                mlist.append([int(plan.tb[s, 0]), gm, False, False])
        # start/stop per gamma
        first = {}
        last = {}
        for j, (tc, gm, _, _) in enumerate(mlist):
            if gm not in first:
                first[gm] = j
            last[gm] = j
        for gm in range(4):
            mlist[first[gm]][2] = True
            mlist[last[gm]][3] = True
        plan.mm.append([tuple(x) for x in mlist])
        plan.m_ofs.append(m_total)
        m_total += len(mlist)
        NM_sg_max = max(NM_sg_max, len(mlist))
    plan.NM = m_total
    plan.NM_sg_max = NM_sg_max

    # per-edge global positions
    q16_s = q16[eord]
    dslot_s = dslot[eord]
    w_s = np.asarray(edge_weight, np.float32)[eord]
    I = (plan.iofs[sg_s, r_s] + tloc) * P + slot  # global idx position per core
    mcol = np.asarray(plan.m_ofs, np.int64)[sg_s] + LK[sg_s, r_s, tloc, gam_s]
    assert (LK[sg_s, r_s, tloc, gam_s] >= 0).all()

    idx_small = np.zeros((C, 16, plan.NT * 8), np.int16)
    idx_small[core_s, I % 16, I // 16] = q16_s.astype(np.int16)
    idx_arr = np.tile(idx_small, (1, 8, 1))
    dst_arr = np.zeros((C, P, plan.NM), np.float16)
    w_arr = np.zeros((C, P, plan.NM), np.float16)
    dst_arr[core_s, slot, mcol] = dslot_s.astype(np.float16)
    w_arr[core_s, slot, mcol] = w_s.astype(np.float16)

    return idx_arr, dst_arr, w_arr, pid, plan


def emulate(x, W, b, idx_arr, dst_arr, w_arr, pid, plan, fp16=True):
    """Numpy emulation of the planned device computation (for validation)."""
    cast = (lambda a: a.astype(np.float16).astype(np.float32)) if fp16 else (lambda a: a)
    x_sh = np.zeros((NPAD, D), np.float32)
    x_sh[pid] = x
    Wc = cast(np.asarray(W, np.float32))
    out = np.zeros((NPAD, D), np.float32)
    h = cast(x_sh) @ Wc[0]
    h = cast(h)  # stored fp16
    qs = np.arange(NQ)
    c_src = qs // (N_PC // 4)
    lq = qs % (N_PC // 4)
    qperm = np.where(lq < CH0Q, c_src * CH0Q + lq,
                     C * CH0Q + c_src * CH1Q + (lq - CH0Q))
    for l in range(L):
        hq = np.empty_like(h.reshape(NQ, 4 * D))
        hq[qperm] = h.reshape(NQ, 4 * D)
        agg = np.zeros((NPAD, D), np.float32)
        for c in range(C):
            for s in range(SG_PC):
                # gather
                msg = np.zeros((plan.T_sg[s] * P, D), np.float32)
                for rr in range(4):
                    Tsr = int(plan.T_sr[s, rr])
                    io = plan.iofs[s, rr]
                    idxs = idx_arr[c, :16, io * 8:(io + Tsr) * 8]
                    flat = idxs.T.reshape(-1)  # (s p) order
                    rows = hq[flat.astype(np.int64), rr * D:(rr + 1) * D]
                    msg[plan.tb[s, rr] * P:(plan.tb[s, rr] + Tsr) * P] = rows
                psums = np.zeros((4, P, D), np.float32)
                mo = plan.m_ofs[s]
                for j, (tc, gm, st, sp) in enumerate(plan.mm[s]):
                    dstc = dst_arr[c, :, mo + j].astype(np.float32)
                    wc = w_arr[c, :, mo + j].astype(np.float32)
                    sel = (dstc[:, None] == np.arange(P)[None, :]) * wc[:, None]
                    psums[gm] += sel.T @ msg[tc * P:(tc + 1) * P]
                for gm in range(4):
                    gg = (c * G_PC) + s * 4 + gm
                    agg[gg * P:(gg + 1) * P] = psums[gm]
        xn = np.maximum(agg + b[l][None, :], 0.0)
        if l < L - 1:
            xn16 = cast(xn)
            h = cast(xn16 @ Wc[l + 1])
        else:
            out = xn
    return out[pid]


def build_nc(plan, queues=1, repeats=1, skip_gather=False, skip_ag=False, skip_sel=False):
    import concourse.bacc as bacc
    import concourse.mybir as mybir
    import concourse.tile as tile

    f32 = mybir.dt.float32
    f16 = mybir.dt.float16
    i16 = mybir.dt.int16
    NT, NM = plan.NT, plan.NM

    nc = bacc.Bacc(
        "TRN2",
        target_bir_lowering=False,
        debug=False,
        num_devices=C,
        num_swdge_queues=queues,
        dynamic_dma_scratch_size=32768,
    )

    xT_in = nc.dram_tensor("xT_in", [P, N_PC], f16, kind="ExternalInput")
    idx_in = nc.dram_tensor("idx_in", [P, NT * 8], i16, kind="ExternalInput")
    dst_in = nc.dram_tensor("dst_in", [P, NM], f16, kind="ExternalInput")
    w_in = nc.dram_tensor("w_in", [P, NM], f16, kind="ExternalInput")
    W_in = nc.dram_tensor("W_in", [L, D, D], f16, kind="ExternalInput")
    b_in = nc.dram_tensor("b_in", [L, P, D], f32, kind="ExternalInput")
    iota_in = nc.dram_tensor("iota_in", [P, P], f16, kind="ExternalInput")
    ident_in = nc.dram_tensor("ident_in", [P, P], f16, kind="ExternalInput")
    x_out = nc.dram_tensor("x_out", [N_PC, D], f32, kind="ExternalOutput")

    with tile.TileContext(nc) as tc:
        with (
            tc.tile_pool(name="const", bufs=1) as cpool,
            tc.tile_pool(name="work", bufs=3) as spool,
            tc.tile_pool(name="selp", bufs=2) as selpool,
            tc.tile_pool(name="msgp", bufs=2) as mpool,
            tc.tile_pool(name="psum", bufs=1, space="PSUM") as ppool,
            tc.tile_pool(name="psum2", bufs=2, space="PSUM") as ppool2,
            tc.tile_pool(name="dram", bufs=1, space="DRAM") as dpool,
        ):
            idx_sb = cpool.tile([P, NT * 8], i16, tag="idx")
            nc.sync.dma_start(out=idx_sb[:], in_=idx_in[:])
            dst_sb = cpool.tile([P, NM], f16, tag="dst")
            nc.sync.dma_start(out=dst_sb[:], in_=dst_in[:])
            w_sb = cpool.tile([P, NM], f16, tag="w")
            nc.sync.dma_start(out=w_sb[:], in_=w_in[:])
            W_sb = cpool.tile([P, L * D], f16, tag="W")
            for l in range(L):
                nc.sync.dma_start(out=W_sb[:, l * D:(l + 1) * D], in_=W_in[l])
            b_sb = cpool.tile([P, L * D], f32, tag="b")
            for l in range(L):
                nc.sync.dma_start(out=b_sb[:, l * D:(l + 1) * D], in_=b_in[l])
            iota_sb = cpool.tile([P, P], f16, tag="iota")
            nc.sync.dma_start(out=iota_sb[:], in_=iota_in[:])
            id_sb = cpool.tile([P, P], f16, tag="ident")
            nc.sync.dma_start(out=id_sb[:], in_=ident_in[:])

            h_loc = [
                dpool.tile([N_PC // 4, 4 * D], f16, tag=f"hloc{l}", name=f"hloc{l}")
                for l in range(L)
            ]
            h_full = [
                dpool.tile([NQ, 4 * D], f16, tag=f"hfull{l}", name=f"hfull{l}")
                for l in range(L)
            ]

            def transform(l, g, xT_t):
                """h_loc[l] rows for group g from xT_t = x^T [feat, node]."""
                psum_h = ppool2.tile([P, P], f32, tag="psum_h")
                nc.tensor.matmul(
                    out=psum_h[:],
                    lhsT=xT_t[:],
                    rhs=W_sb[:, l * D:(l + 1) * D],
                    start=True,
                    stop=True,
                )
                h_sb = spool.tile([P, P], f16, tag="h_sb")
                nc.vector.tensor_copy(out=h_sb[:], in_=psum_h[:])
                nc.sync.dma_start(
                    out=h_loc[l][g * 32:(g + 1) * 32, :], in_=h_sb[:]
                )

            def all_gather(l, ch):
                if skip_ag:
                    if ch == 0:
                        nc.sync.dma_start(out=h_full[l][0:N_PC // 4, :],
                                          in_=h_loc[l][:, :])
                    return
                if ch == 0:
                    ins_, outs_ = h_loc[l][0:CH0Q, :], h_full[l][0:C * CH0Q, :]
                else:
                    ins_, outs_ = h_loc[l][CH0Q:, :], h_full[l][C * CH0Q:, :]
                nc.gpsimd.collective_compute(
                    "AllGather",
                    mybir.AluOpType.bypass,
                    replica_groups=[list(range(C))],
                    ins=[ins_],
                    outs=[outs_],
                )

            for _rep in range(repeats):
              for g in range(G_PC):
                xT_t = spool.tile([P, P], f16, tag="xT")
                nc.sync.dma_start(out=xT_t[:], in_=xT_in[:, g * P:(g + 1) * P])
                transform(0, g, xT_t)
                if g == CH0_G - 1:
                    all_gather(0, 0)
              all_gather(0, 1)

              for l in range(L):
                last = l == L - 1
                for s in range(SG_PC):
                    msg = mpool.tile([P, plan.T_sg_max * P], f16, tag="msg")
                    for ci, (lc, io, nt_, rr) in enumerate(plan.calls[s]):
                        out3 = msg[:, lc * P:(lc + nt_) * P].rearrange(
                            "p (c f) -> p c f", f=P
                        )
                        if skip_gather:
                            base = ((s * 31 + ci) * 977) % (NQ - nt_ * 32)
                            nc.sync.dma_start(
                                out=msg[:, lc * P:(lc + nt_) * P],
                                in_=h_full[l][base:base + nt_ * 32, :])
                            continue
                        nc.gpsimd.dma_gather(
                            out3,
                            h_full[l][:, rr * D:(rr + 1) * D],
                            idx_sb[:, io * 8:(io + nt_) * 8],
                            nt_ * P,
                            nt_ * P,
                            D,
                            elem_step=4 * D,
                            queue_num=ci % queues,
                        )
                    nm = len(plan.mm[s])
                    mo = plan.m_ofs[s]
                    sel = selpool.tile([P, plan.NM_sg_max * P], f16, tag="sel")
                    sel3 = sel[:, :nm * P].rearrange("p (m j) -> p m j", j=P)
                    dst3 = dst_sb[:, mo:mo + nm].to_broadcast([P, nm, P])
                    w3 = w_sb[:, mo:mo + nm].to_broadcast([P, nm, P])
                    iota3 = (
                        iota_sb[:]
                        .rearrange("p (o j) -> p o j", o=1)
                        .to_broadcast([P, nm, P])
                    )
                    nc.vector.tensor_tensor(
                        out=sel3, in0=dst3, in1=iota3, op=mybir.AluOpType.is_equal
                    )
                    if not skip_sel:
                        nc.vector.tensor_tensor(
                            out=sel3, in0=sel3, in1=w3, op=mybir.AluOpType.mult
                        )
                    paggs = [
                        ppool.tile([P, P], f32, tag=f"agg{gm}", name=f"agg{gm}")
                        for gm in range(4)
                    ]
                    for j, (tc_, gm, st, sp) in enumerate(plan.mm[s]):
                        nc.tensor.matmul(
                            out=paggs[gm][:],
                            lhsT=sel[:, j * P:(j + 1) * P],
                            rhs=msg[:, tc_ * P:(tc_ + 1) * P],
                            start=st,
                            stop=sp,
                        )
                    for gm in range(4):
                        g = s * 4 + gm
                        if last:
                            xo = spool.tile([P, P], f32, tag="xo")
                            nc.vector.tensor_tensor(
                                out=xo[:],
                                in0=paggs[gm][:],
                                in1=b_sb[:, l * D:(l + 1) * D],
                                op=mybir.AluOpType.add,
                            )
                            nc.vector.tensor_scalar_max(xo[:], xo[:], 0.0)
                            nc.sync.dma_start(
                                out=x_out[g * P:(g + 1) * P, :], in_=xo[:]
                            )
                        else:
                            xnew = spool.tile([P, P], f16, tag="xnew")
                            nc.vector.tensor_tensor(
                                out=xnew[:],
                                in0=paggs[gm][:],
                                in1=b_sb[:, l * D:(l + 1) * D],
                                op=mybir.AluOpType.add,
                            )
                            nc.vector.tensor_scalar_max(xnew[:], xnew[:], 0.0)
                            psum_t = ppool2.tile([P, P], f16, tag="psum_t")
                            nc.tensor.transpose(
                                out=psum_t[:], in_=xnew[:], identity=id_sb[:]
                            )
                            xT_t = spool.tile([P, P], f16, tag="xT")
                            nc.vector.tensor_copy(out=xT_t[:], in_=psum_t[:])
                            transform(l + 1, g, xT_t)
                    if not last and s == CH0_G // 4 - 1:
                        all_gather(l + 1, 0)
                if not last:
                    all_gather(l + 1, 1)

    nc.compile()
    return nc


def make_host_inputs(x, edge_src, edge_dst, edge_weight, W, b):
    idx_arr, dst_arr, w_arr, pid, plan = partition_graph(
        edge_src, edge_dst, edge_weight
    )
    x_sh = np.zeros((NPAD, D), np.float32)
    x_sh[pid] = np.asarray(x, np.float32)
    xT = (
        x_sh.reshape(C, N_PC, D)
        .transpose(0, 2, 1)
        .astype(np.float16)
        .copy()
    )
    b_rep = np.broadcast_to(
        np.asarray(b, np.float32)[:, None, :], (L, P, D)
    ).copy()
    iota = np.broadcast_to(
        np.arange(P, dtype=np.float16), (P, P)
    ).copy()
    ident = np.eye(P, dtype=np.float16)
    W16 = np.asarray(W, np.float32).astype(np.float16)
    in_maps = [
        {
            "xT_in": xT[c],
            "idx_in": idx_arr[c],
            "dst_in": dst_arr[c],
            "w_in": w_arr[c],
            "W_in": W16,
            "b_in": b_rep,
            "iota_in": iota,
            "ident_in": ident,
        }
        for c in range(C)
    ]
    return in_maps, pid, plan


def unshard_output(results, pid):
    full = np.concatenate([results[c]["x_out"] for c in range(C)], axis=0)
    return full[pid]


QUEUES = 4


def run(x, edge_src, edge_dst, edge_weight, W, b, queues=None):
    if queues is None:
        queues = QUEUES
    from concourse.bass_utils import run_bass_kernel_spmd

    x = np.asarray(x, np.float32)
    edge_src = np.asarray(edge_src, np.int32)
    edge_dst = np.asarray(edge_dst, np.int32)
    edge_weight = np.asarray(edge_weight, np.float32)
    in_maps, pid, plan = make_host_inputs(x, edge_src, edge_dst, edge_weight, W, b)
    nc = build_nc(plan, queues=queues)
    res = run_bass_kernel_spmd(nc, in_maps, core_ids=list(range(C)))
    return unshard_output(res.results, pid), res


def kernel(x, edge_src, edge_dst, edge_weight, W, b):
    out, _ = run(x, edge_src, edge_dst, edge_weight, W, b)
    return out
